# revision 1
# baseline (speedup 1.0000x reference)
"""Trainium2 Bass kernel for the HardResetSSMBlock problem.

y = silu(x @ W1 + b1) @ W2 + b2, masked per frame, with a periodic decay
scale on frames where (t+1) % 10 == 0.

Strategy: the mask zeroes ~half the output tokens, and the op is
stateless per token, so the host packs only the unmasked tokens into a
dense stream (pure data movement -- all FLOPs stay on device), splits
it evenly across 8 NeuronCores, and scatters the device results back
into a zero-filled output. This halves both HBM traffic and compute.
All device HBM traffic is bf16 (PSUM accumulation stays f32; ~4e-3
max rel err). The remaining per-token scale fed to the device is the
decay factor at each kept token's original position.

Device pipeline per 1024-token group (4096-token supertiles = 4 groups,
1MB bf16 DMAs each way):
  MM1 (W1 stationary, X moving, 2x N=512) into a 2-bank PSUM group ->
  Silu(+b1) on ACT writing bf16 -> MM2 with 128-token H^T chunks
  stationary (FWL, bf16) so Y lands token-major, two 512-token halves
  into 1-bank PSUM buffers -> decay scale via per-partition broadcast
  tensor_mul on DVE, f32 PSUM -> bf16 SBUF -> 1MB DMA out per supertile
  (last one split across both HWDGE queues to shorten the drain).
MM2 is skewed two groups behind MM1 (PSUM: 3x2 + 2x1 banks) so the ACT
latency is hidden from the PE; 30 junk matmuls + a dummy activation
during the DMA fill warm the PE clock (HAM) and preload the silu
spline tables.

The device kernel is compiled for ceil(max_core_tokens/4096) tiles and
cached per tile count, so any mask density (including all-ones) works.
"""

import numpy as np

B, S, D = 16, 16384, 128
RESET_PERIOD = 10
DECAY_FACTOR = 0.1
N_CORES = 8
TILE_TOK = 4096
CH = TILE_TOK // 128  # 32 chunks of 128 tokens
GRP = 1024  # tokens per PSUM group (2 banks)
G_PER_TILE = TILE_TOK // GRP  # 4
PREFETCH = 3
SKEW = 2  # groups of MM1->ACT lead before MM2 drains

# Silu on hardware; CoreSim lacks it, so sim tests may override (e.g. Sigmoid)
ACT_FUNC = "Silu"

_CACHE = {}


def _build_nc(n_tiles):
    import concourse.bacc as bacc
    import concourse.tile as tile
    from concourse import mybir
    from concourse.bass import AP

    f32 = mybir.dt.float32
    bf16 = mybir.dt.bfloat16
    n_grp = n_tiles * G_PER_TILE
    n_chunks = n_tiles * CH
    act_fn = getattr(mybir.ActivationFunctionType, ACT_FUNC)

    nc = bacc.Bacc()
    xt_d = nc.dram_tensor(
        "x_t", [n_tiles, 128, TILE_TOK], bf16, kind="ExternalInput"
    )
    st_d = nc.dram_tensor("s_t", [128, n_chunks], f32, kind="ExternalInput")
    w1_d = nc.dram_tensor("w1", [D, D], bf16, kind="ExternalInput")
    w2_d = nc.dram_tensor("w2", [D, D], bf16, kind="ExternalInput")
    b1_d = nc.dram_tensor("b1", [D, 1], f32, kind="ExternalInput")
    # partition-major output tiles: y_t[t, p, c, d] = y[(t*CH + c)*128 + p, d]
    # (host un-permutes; this makes the out-DMA write 4KB contiguous runs)
    y_d = nc.dram_tensor(
        "y_t", [n_tiles, 128, CH, D], bf16, kind="ExternalOutput"
    )

    with tile.TileContext(nc) as tc:
        with (
            tc.tile_pool(name="const", bufs=1) as constp,
            tc.tile_pool(name="data", bufs=1) as datap,
            tc.tile_pool(name="ps_ht", bufs=SKEW + 1, space="PSUM") as ps_htp,
            tc.tile_pool(name="ps_y", bufs=2, space="PSUM") as ps_yp,
        ):
            # --- warmup: keep the PE busy during the DMA fill so HAM
            # unthrottles (2.4 GHz) before the first real matmul, and
            # preload the silu spline tables on ACT.
            junk = constp.tile([128, 128], bf16, name="junk")
            nc.vector.memset(junk[:], 0.0)
            p_j = ps_yp.tile([128, GRP // 256, 128], f32, name="p_j")
            for _ in range(30):
                nc.tensor.matmul(
                    p_j[:, 0, :], junk[:], junk[:], start=True, stop=True
                )

            b1_s = constp.tile([128, 1], f32)
            nc.sync.dma_start(b1_s[:], b1_d[:])
            w1_s = constp.tile([128, 128], bf16)
            nc.sync.dma_start(w1_s[:], w1_d[:])
            w2_s = constp.tile([128, 128], bf16)
            nc.scalar.dma_start(w2_s[:], w2_d[:])
            st_s = constp.tile([128, n_chunks], f32)

            x_tiles = [None] * n_tiles
            y_tiles = [None] * n_tiles
            h_grp = [None] * n_grp

            def in_dma(t, split=False):
                if t >= n_tiles:
                    return
                x_tiles[t] = datap.tile(
                    [128, TILE_TOK], bf16, name="s_xt", bufs=PREFETCH + 1
                )
                if split:
                    nc.sync.dma_start(
                        x_tiles[t][:, :GRP], xt_d[t, :, :GRP]
                    )
                    nc.scalar.dma_start(
                        x_tiles[t][:, GRP:], xt_d[t, :, GRP:]
                    )
                    return
                else:
                    eng = nc.sync if t % 2 == 0 else nc.scalar
                    eng.dma_start(x_tiles[t][:], xt_d[t])

            in_dma(0, split=True)
            nc.scalar.dma_start(st_s[:], st_d[:])
            for t in range(1, PREFETCH):
                in_dma(t)

            # preload the silu spline tables (ACT_TABLE_LOAD attaches to
            # the first activation on the ring) without blocking the
            # scalar ring's DMA issues above
            actwarm = constp.tile([128, 1], f32, name="actwarm")
            nc.scalar.activation(
                actwarm[:], junk[:, 0:1], act_fn, bias=b1_s[:], scale=1.0
            )

            # software-pipelined: PE order is MM1(0), MM1(1), MM1(2),
            # MM2(0), MM1(3), MM2(1), ... so ACT(g) latency is hidden.
            for g in range(n_grp + SKEW):
                if g < n_grp:
                    t = g // G_PER_TILE
                    if g % G_PER_TILE == 0:
                        in_dma(t + PREFETCH)
                        y_tiles[t] = datap.tile(
                            [128, CH, D], bf16, name="s_y", bufs=2
                        )
                    s_xt = x_tiles[t]
                    off = (g % G_PER_TILE) * GRP
                    ps = ps_htp.tile([128, GRP], f32)
                    for h in range(GRP // 512):
                        hs = slice(off + h * 512, off + (h + 1) * 512)
                        nc.tensor.matmul(
                            ps[:, h * 512:(h + 1) * 512], w1_s[:],
                            s_xt[:, hs], start=True, stop=True,
                        )
                    h_grp[g] = datap.tile([128, GRP], bf16, name="s_h", bufs=4)
                    nc.scalar.activation(
                        h_grp[g][:], ps[:], act_fn, bias=b1_s[:], scale=1.0
                    )

                if g >= SKEW:
                    gp = g - SKEW
                    tp = gp // G_PER_TILE
                    for half in range(2):
                        c0 = (gp % G_PER_TILE) * (GRP // 128) + half * 4
                        p_y = ps_yp.tile([128, 4, 128], f32, name="p_j")
                        for c in range(4):
                            cc = half * 4 + c
                            nc.tensor.matmul(
                                p_y[:, c, :],
                                h_grp[gp][:, cc * 128:(cc + 1) * 128], w2_s,
                                start=True, stop=True,
                            )
                        s_slice = st_s[:, tp * CH + c0:tp * CH + c0 + 4]
                        s_bcast = AP(
                            tensor=s_slice.tensor,
                            offset=s_slice.offset,
                            ap=list(s_slice.ap) + [[0, 128]],
                        )  # [128, 4, 128] with stride-0 feature dim
                        nc.vector.tensor_mul(
                            y_tiles[tp][:, c0:c0 + 4, :], p_y[:], s_bcast
                        )
                    if gp % G_PER_TILE == G_PER_TILE - 1:
                        if tp == n_tiles - 1:
                            # split the last write across two queues to
                            # shorten the final drain
                            half_ch = CH // 2
                            nc.scalar.dma_start(
                                y_d[tp, :, :half_ch], y_tiles[tp][:, :half_ch]
                            )
                            nc.sync.dma_start(
                                y_d[tp, :, half_ch:], y_tiles[tp][:, half_ch:]
                            )
                        else:
                            nc.sync.dma_start(y_d[tp], y_tiles[tp][:])

    nc.finalize()
    return nc


def _get_nc(n_tiles):
    key = ("nc", n_tiles)
    if key not in _CACHE:
        _CACHE[key] = _build_nc(n_tiles)
    return _CACHE[key]


def kernel(x, mask, W1, b1, W2, b2, _trace=False):
    from ml_dtypes import bfloat16
    from concourse.bass_utils import run_bass_kernel_spmd

    x = np.asarray(x, dtype=np.float32)
    mask = np.asarray(mask)
    W1b = np.ascontiguousarray(np.asarray(W1, dtype=np.float32)).astype(
        bfloat16
    )
    W2b = np.ascontiguousarray(np.asarray(W2, dtype=np.float32)).astype(
        bfloat16
    )
    b1v = np.asarray(b1, dtype=np.float32).reshape(D, 1)
    b2 = np.asarray(b2, dtype=np.float32)

    t = np.arange(S)
    decay = np.where((t + 1) % RESET_PERIOD == 0, DECAY_FACTOR, 1.0).astype(
        np.float32
    )

    # pack unmasked tokens into a dense stream, split evenly over cores
    mask_flat = mask.reshape(-1)
    idx = np.flatnonzero(mask_flat)
    K = idx.size
    out_flat = np.zeros((B * S, D), dtype=np.float32)
    if K:
        k8 = -(-K // N_CORES)
        n_tiles = max(1, -(-k8 // TILE_TOK))
        cap = n_tiles * TILE_TOK
        tot = cap * N_CORES

        xp = np.zeros((tot, D), dtype=bfloat16)
        xp[:K] = x.reshape(B * S, D)[idx]
        sp = np.zeros(tot, dtype=np.float32)
        sp[:K] = np.broadcast_to(decay[None, :], (B, S)).reshape(-1)[idx]

        # feature-major tiles: [core, n_tiles, 128(d), TILE_TOK]
        x_t_all = np.ascontiguousarray(
            xp.reshape(N_CORES, n_tiles, TILE_TOK, D).transpose(0, 1, 3, 2)
        )
        s_all = sp.reshape(N_CORES, cap // 128, 128)

        in_maps = []
        for c in range(N_CORES):
            s_t = np.ascontiguousarray(s_all[c].T)  # [128, n_chunks]
            in_maps.append(
                {
                    "x_t": x_t_all[c],
                    "s_t": s_t,
                    "w1": W1b,
                    "w2": W2b,
                    "b1": b1v,
                }
            )

        nc = _get_nc(n_tiles)
        res = run_bass_kernel_spmd(
            nc, in_maps, list(range(N_CORES)), trace=_trace
        )
        if _trace:
            _CACHE["last_results"] = res
        # y_t[t, p, c, d] -> packed token (t*CH + c)*128 + p
        yp = np.stack(
            [np.asarray(res.results[c]["y_t"]) for c in range(N_CORES)]
        )
        yp = (
            yp.transpose(0, 1, 3, 2, 4)
            .astype(np.float32)
            .reshape(N_CORES * cap, D)
        )
        out_flat[idx] = yp[:K]

    out = out_flat.reshape(B, S, D)
    if np.any(b2):
        # device computes (h @ W2) * s; the masked/decayed bias is added here
        s = mask.astype(np.float32) * decay[None, :]
        out = out + s[:, :, None] * b2[None, None, :]
    return out



# revision 2
# speedup vs baseline: 1.2393x; 1.2393x over previous
"""Trainium2 Bass kernel for the HardResetSSMBlock problem.

y = silu(x @ W1 + b1) @ W2 + b2, masked per frame, with a periodic decay
scale on frames where (t+1) % 10 == 0.

Strategy: the mask zeroes ~half the output tokens and the op is stateless
per token, so the host packs only the unmasked tokens into a dense
stream (pure data movement -- all FLOPs stay on device) and splits it
evenly across 8 NeuronCores. Tokens are rebalanced across cores so every
core gets exactly the same number of normal and decayed tokens: the
stream is [normal tokens | decayed tokens] with ONE boundary column
shared by all cores (SPMD single program). The decay is folded into the
weights: decayed tokens use W2' = 0.1*W2 as the MM2 stationary operand,
so no per-token scale tensor and no broadcast multiply exist on device.

Device dataflow per 1024-token group (feature-major, 2 PSUM banks,
4-deep rotation over all 8 banks):
  MM1 (W1 stationary bf16, x moving fp8-e3m4, 2x N=512) -> PSUM
  -> Silu(+b1) on ACT -> h bf16 SBUF
  -> MM2 (W2/W2' stationary, h moving, N<=512 pieces split at the
     decay boundary) -> SAME PSUM banks (reused after ACT read)
  -> DVE tensor_copy PSUM->SBUF bf16
  -> 1MB out-DMA per 4096-token tile on the gpsimd (SWDGE) ring.
In-DMAs (x tiles fp8, 512KB each) all ride the sync (HWDGE) ring,
issued up front; the ACT ring issues no DMAs at all so the scalar
engine spends every cycle on silu. Input x is fp8-e3m4 (4 mantissa
bits): measured end-to-end rel err 1.4e-2 vs the 2e-2 gate; this
halves input HBM traffic. Output stays bf16.

The device kernel is compiled per (n_tiles, boundary) and cached, so
any mask density works.
"""

import numpy as np

B, S, D = 16, 16384, 128
RESET_PERIOD = 10
DECAY_FACTOR = 0.1
N_CORES = 8
TILE_TOK = 4096
GRP = 1024  # tokens per PSUM group (2 banks)
G_PER_TILE = TILE_TOK // GRP  # 4
SKEW = 2  # groups of MM1->ACT lead before MM2 drains
N_JUNK = 14  # PE warmup matmuls during the DMA fill

ACT_FUNC = "Silu"

_CACHE = {}


def _mm2_pieces(g, bnd):
    """Sub-matmul splits for group g: (c0, c1, use_decay_weights)."""
    base = g * GRP
    pieces = []
    for h in range(GRP // 512):
        c0, c1 = h * 512, (h + 1) * 512
        g0, g1 = base + c0, base + c1
        if g1 <= bnd or g0 >= bnd:
            pieces.append((c0, c1, g0 >= bnd))
        else:
            pieces.append((c0, bnd - base, False))
            pieces.append((bnd - base, c1, True))
    return pieces


def _build_nc(n_tiles, bnd):
    import concourse.bacc as bacc
    import concourse.tile as tile
    from concourse import mybir

    f32 = mybir.dt.float32
    bf16 = mybir.dt.bfloat16
    f8 = mybir.dt.float8e3
    n_grp = n_tiles * G_PER_TILE
    act_fn = getattr(mybir.ActivationFunctionType, ACT_FUNC)

    nc = bacc.Bacc()
    xt_d = nc.dram_tensor("x_t", [n_tiles, 128, TILE_TOK], f8, kind="ExternalInput")
    w1_d = nc.dram_tensor("w1", [D, D], bf16, kind="ExternalInput")
    w2_d = nc.dram_tensor("w2", [D, D], bf16, kind="ExternalInput")
    w2p_d = nc.dram_tensor("w2p", [D, D], bf16, kind="ExternalInput")
    b1_d = nc.dram_tensor("b1", [D, 1], f32, kind="ExternalInput")
    y_d = nc.dram_tensor("y_t", [n_tiles, 128, TILE_TOK], bf16, kind="ExternalOutput")

    with tile.TileContext(nc) as tc:
        with (
            tc.tile_pool(name="const", bufs=1) as constp,
            tc.tile_pool(name="data", bufs=1) as datap,
            tc.tile_pool(name="ps", bufs=4, space="PSUM") as psp,
        ):
            # constants / weights on the sync ring, ahead of the x tiles
            b1_s = constp.tile([128, 1], f32)
            nc.sync.dma_start(b1_s[:], b1_d[:])
            w1_s = constp.tile([128, 128], bf16)
            nc.sync.dma_start(w1_s[:], w1_d[:])

            x_tiles = [None] * n_tiles
            for t in range(n_tiles):
                x_tiles[t] = datap.tile(
                    [128, TILE_TOK], f8, name="s_xt", bufs=n_tiles
                )
            nc.sync.dma_start(x_tiles[0][:], xt_d[0])
            w2_s = constp.tile([128, 128], bf16)
            nc.sync.dma_start(w2_s[:], w2_d[:])
            w2p_s = constp.tile([128, 128], bf16)
            nc.sync.dma_start(w2p_s[:], w2p_d[:])
            for t in range(1, n_tiles):
                nc.sync.dma_start(x_tiles[t][:], xt_d[t])

            # --- warmup: junk matmuls keep the PE HAM window busy during
            # the DMA fill; a dummy activation preloads the silu tables.
            junk = constp.tile([128, 128], bf16, name="junk")
            nc.vector.memset(junk[:], 0.0)
            p_j = psp.tile([128, GRP], f32, name="ps")
            for _ in range(N_JUNK):
                nc.tensor.matmul(
                    p_j[:, 0:128], junk[:], junk[:], start=True, stop=True
                )
            actwarm = constp.tile([128, 1], f32, name="actwarm")
            nc.scalar.activation(actwarm[:], junk[:, 0:1], act_fn, scale=1.0)

            y_tiles = [None] * n_tiles
            h_grp = [None] * n_grp
            ps_grp = [None] * n_grp

            for g in range(n_grp + SKEW):
                if g < n_grp:
                    t = g // G_PER_TILE
                    if g % G_PER_TILE == 0:
                        y_tiles[t] = datap.tile(
                            [128, TILE_TOK], bf16, name="s_y", bufs=2
                        )
                    off = (g % G_PER_TILE) * GRP
                    ps = psp.tile([128, GRP], f32, name="ps")
                    ps_grp[g] = ps
                    xs = x_tiles[t]
                    for h in range(GRP // 512):
                        sl = slice(h * 512, (h + 1) * 512)
                        nc.tensor.matmul(
                            ps[:, sl], w1_s[:], xs[:, off + h * 512:off + (h + 1) * 512],
                            start=True, stop=True,
                        )
                    h_grp[g] = datap.tile([128, GRP], bf16, name="s_h", bufs=4)
                    nc.scalar.activation(
                        h_grp[g][:], ps[:], act_fn, bias=b1_s[:], scale=1.0
                    )

                if g >= SKEW:
                    gp = g - SKEW
                    tp = gp // G_PER_TILE
                    offp = (gp % G_PER_TILE) * GRP
                    ps = ps_grp[gp]
                    for c0, c1, dec in _mm2_pieces(gp, bnd):
                        w_s = w2p_s if dec else w2_s
                        nc.tensor.matmul(
                            ps[:, c0:c1], w_s[:], h_grp[gp][:, c0:c1],
                            start=True, stop=True,
                        )
                    nc.vector.tensor_copy(
                        y_tiles[tp][:, offp:offp + GRP], ps[:]
                    )
                    if gp % G_PER_TILE == G_PER_TILE - 1:
                        if tp == n_tiles - 1:
                            # split the final drain across both rings
                            half = TILE_TOK // 2
                            nc.gpsimd.dma_start(
                                y_d[tp, :, :half], y_tiles[tp][:, :half]
                            )
                            nc.sync.dma_start(
                                y_d[tp, :, half:], y_tiles[tp][:, half:]
                            )
                        else:
                            nc.gpsimd.dma_start(y_d[tp], y_tiles[tp][:])

    nc.finalize()
    return nc


def _get_nc(n_tiles, bnd):
    key = ("nc", n_tiles, bnd)
    if key not in _CACHE:
        _CACHE[key] = _build_nc(n_tiles, bnd)
    return _CACHE[key]


def kernel(x, mask, W1, b1, W2, b2, _trace=False):
    from ml_dtypes import bfloat16, float8_e3m4
    from concourse.bass_utils import run_bass_kernel_spmd

    x = np.asarray(x, dtype=np.float32)
    mask = np.asarray(mask)
    W1b = np.ascontiguousarray(np.asarray(W1, dtype=np.float32)).astype(bfloat16)
    W2f = np.ascontiguousarray(np.asarray(W2, dtype=np.float32))
    W2b = W2f.astype(bfloat16)
    W2pb = (W2f * DECAY_FACTOR).astype(bfloat16)
    b1v = np.asarray(b1, dtype=np.float32).reshape(D, 1)
    b2 = np.asarray(b2, dtype=np.float32)

    t = np.arange(S)
    dec_frame = (t + 1) % RESET_PERIOD == 0

    mask_flat = mask.reshape(-1)
    dec_flat = np.broadcast_to(dec_frame[None, :], (B, S)).reshape(-1)
    idx = np.flatnonzero(mask_flat)
    K = idx.size
    out_flat = np.zeros((B * S, D), dtype=np.float32)
    if K:
        sel_dec = dec_flat[idx]
        idx_norm = idx[~sel_dec]
        idx_dec = idx[sel_dec]
        n_norm = -(-idx_norm.size // N_CORES)
        n_dec = -(-idx_dec.size // N_CORES)
        bnd = n_norm
        t_req = n_norm + n_dec
        n_tiles = max(1, -(-t_req // TILE_TOK))
        T = n_tiles * TILE_TOK

        # per-core slot -> source token index (-1 = padding)
        src = np.full((N_CORES, T), -1, dtype=np.int64)
        for c in range(N_CORES):
            a = idx_norm[c * n_norm:(c + 1) * n_norm]
            src[c, :a.size] = a
            d = idx_dec[c * n_dec:(c + 1) * n_dec]
            src[c, bnd:bnd + d.size] = d
        valid = src >= 0

        xp = np.zeros((N_CORES, T, D), dtype=np.float32)
        xp[valid] = x.reshape(B * S, D)[src[valid]]
        x8 = xp.astype(float8_e3m4)
        # feature-major tiles: [core, n_tiles, 128(d), TILE_TOK]
        x_t_all = np.ascontiguousarray(
            x8.reshape(N_CORES, n_tiles, TILE_TOK, D).transpose(0, 1, 3, 2)
        )

        in_maps = []
        for c in range(N_CORES):
            in_maps.append(
                {
                    "x_t": x_t_all[c],
                    "w1": W1b,
                    "w2": W2b,
                    "w2p": W2pb,
                    "b1": b1v,
                }
            )

        nc = _get_nc(n_tiles, bnd)
        res = run_bass_kernel_spmd(nc, in_maps, list(range(N_CORES)), trace=_trace)
        if _trace:
            _CACHE["last_results"] = res
        yp = np.stack(
            [np.asarray(res.results[c]["y_t"]) for c in range(N_CORES)]
        )  # [cores, n_tiles, 128, TILE_TOK] bf16
        yp = (
            yp.transpose(0, 1, 3, 2)
            .astype(np.float32)
            .reshape(N_CORES, T, D)
        )
        out_flat[src[valid]] = yp[valid]

    out = out_flat.reshape(B, S, D)
    if np.any(b2):
        # device computes h @ W2(/W2'); the masked/decayed bias lands here
        scale = np.where(dec_frame, DECAY_FACTOR, 1.0).astype(np.float32)
        s = mask.astype(np.float32) * scale[None, :]
        out = out + s[:, :, None] * b2[None, None, :]
    return out


# revision 3
# speedup vs baseline: 1.2504x; 1.0090x over previous
"""Trainium2 Bass kernel for the HardResetSSMBlock problem.

y = silu(x @ W1 + b1) @ W2 + b2, masked per frame, with a periodic decay
scale on frames where (t+1) % 10 == 0.

Strategy: the mask zeroes ~half the output tokens and the op is stateless
per token, so the host packs only the unmasked tokens into a dense
stream (pure data movement -- all FLOPs stay on device) and splits it
evenly across 8 NeuronCores. Tokens are rebalanced across cores so every
core gets exactly the same number of normal and decayed tokens: the
stream is [normal tokens | decayed tokens] with ONE boundary column
shared by all cores (SPMD single program). The decay is folded into the
weights: decayed tokens use W2' = 0.1*W2 as the MM2 stationary operand,
so no per-token scale tensor and no broadcast multiply exist on device.

Device dataflow per 1024-token group (feature-major, 2 PSUM banks,
4-deep rotation over all 8 banks):
  MM1 (W1 stationary bf16, x moving fp8-e3m4, 2x N=512) -> PSUM
  -> Silu(+b1) on ACT -> h bf16 SBUF
  -> MM2 (W2/W2' stationary, h moving, N<=512 pieces split at the
     decay boundary) -> SAME PSUM banks (reused after ACT read)
  -> DVE tensor_copy PSUM->SBUF bf16
  -> out-DMA per 4096-token tile on the gpsimd (SWDGE) ring.

DMA layout notes (the ~290ns/descriptor floor dominates): x and y are
[128, T] in DRAM so per-partition runs are 4-12KB; x rides the sync
ring as two transfers ([0:4096] for fast first-group availability,
[4096:T] at 12KB/descriptor line rate); the three weight matrices are
packed into ONE [128, 384] bf16 tensor on the scalar ring so no tiny
descriptors sit ahead of x; b1 is an on-device zero constant when the
input b1 is all zeros (the graded case). Input x is fp8-e3m4
(4 mantissa bits): measured end-to-end rel err 1.39e-2 vs the 2e-2
gate, halving input HBM traffic. Output stays bf16.

The device kernel is compiled per (n_tiles, boundary, has_bias) and
cached, so any mask density works.
"""

import numpy as np

B, S, D = 16, 16384, 128
RESET_PERIOD = 10
DECAY_FACTOR = 0.1
N_CORES = 8
TILE_TOK = 4096
GRP = 1024  # tokens per PSUM group (2 banks)
G_PER_TILE = TILE_TOK // GRP  # 4
SKEW = 2  # groups of MM1->ACT lead before MM2 drains
N_JUNK = 30  # PE warmup matmuls during the DMA fill

ACT_FUNC = "Silu"

_CACHE = {}


def _mm2_pieces(g, bnd):
    """Sub-matmul splits for group g: (c0, c1, use_decay_weights)."""
    base = g * GRP
    pieces = []
    for h in range(GRP // 512):
        c0, c1 = h * 512, (h + 1) * 512
        g0, g1 = base + c0, base + c1
        if g1 <= bnd or g0 >= bnd:
            pieces.append((c0, c1, g0 >= bnd))
        else:
            pieces.append((c0, bnd - base, False))
            pieces.append((bnd - base, c1, True))
    return pieces


def _build_nc(n_tiles, bnd, has_bias):
    import concourse.bacc as bacc
    import concourse.tile as tile
    from concourse import mybir

    f32 = mybir.dt.float32
    bf16 = mybir.dt.bfloat16
    f8 = mybir.dt.float8e3
    T = n_tiles * TILE_TOK
    n_grp = n_tiles * G_PER_TILE
    act_fn = getattr(mybir.ActivationFunctionType, ACT_FUNC)

    nc = bacc.Bacc()
    xt_d = nc.dram_tensor("x_t", [128, T], f8, kind="ExternalInput")
    wp_d = nc.dram_tensor("wp", [D, 3 * D], bf16, kind="ExternalInput")
    if has_bias:
        b1_d = nc.dram_tensor("b1", [D, 1], f32, kind="ExternalInput")
    y_d = nc.dram_tensor("y_t", [128, T], bf16, kind="ExternalOutput")

    with tile.TileContext(nc) as tc:
        with (
            tc.tile_pool(name="const", bufs=1) as constp,
            tc.tile_pool(name="data", bufs=1) as datap,
            tc.tile_pool(name="ps", bufs=4, space="PSUM") as psp,
        ):
            # x input: first tile alone for fast first-group availability,
            # the rest as one line-rate transfer. Both on the sync ring,
            # ahead of everything else on that ring.
            x_a = datap.tile([128, TILE_TOK], f8, name="s_xa")
            nc.sync.dma_start(x_a[:], xt_d[:, 0:TILE_TOK])
            if n_tiles > 1:
                x_b = datap.tile([128, T - TILE_TOK], f8, name="s_xb")
                nc.sync.dma_start(x_b[:], xt_d[:, TILE_TOK:T])

            # weights packed [w1 | w2 | w2'] ride the scalar ring: one
            # issue before the activations start, no tiny descriptors on
            # the sync ring ahead of x.
            wp_s = constp.tile([128, 3 * D], bf16)
            nc.scalar.dma_start(wp_s[:], wp_d[:])
            w1_s = wp_s[:, 0:D]
            w2_s = wp_s[:, D:2 * D]
            w2p_s = wp_s[:, 2 * D:3 * D]
            if has_bias:
                b1_s = constp.tile([128, 1], f32)
                nc.scalar.dma_start(b1_s[:], b1_d[:])
                bias = b1_s[:]
            else:
                bias = 0.0

            # --- warmup: junk matmuls keep the PE HAM window busy during
            # the DMA fill; a dummy activation preloads the silu tables.
            junk = constp.tile([128, 128], bf16, name="junk")
            nc.vector.memset(junk[:], 0.0)
            p_j = psp.tile([128, GRP], f32, name="ps")
            for _ in range(N_JUNK):
                nc.tensor.matmul(
                    p_j[:, 0:128], junk[:], junk[:], start=True, stop=True
                )
            actwarm = constp.tile([128, 1], f32, name="actwarm")
            nc.scalar.activation(actwarm[:], junk[:, 0:1], act_fn, scale=1.0)

            y_tiles = [None] * n_tiles
            h_grp = [None] * n_grp
            ps_grp = [None] * n_grp

            def x_cols(g):
                if g < G_PER_TILE:
                    return x_a[:, g * GRP:(g + 1) * GRP]
                off = g * GRP - TILE_TOK
                return x_b[:, off:off + GRP]

            for g in range(n_grp + SKEW):
                if g < n_grp:
                    t = g // G_PER_TILE
                    if g % G_PER_TILE == 0:
                        y_tiles[t] = datap.tile(
                            [128, TILE_TOK], bf16, name="s_y", bufs=2
                        )
                    xs = x_cols(g)
                    ps = psp.tile([128, GRP], f32, name="ps")
                    ps_grp[g] = ps
                    for h in range(GRP // 512):
                        sl = slice(h * 512, (h + 1) * 512)
                        nc.tensor.matmul(
                            ps[:, sl], w1_s, xs[:, sl], start=True, stop=True
                        )
                    h_grp[g] = datap.tile([128, GRP], bf16, name="s_h", bufs=4)
                    nc.scalar.activation(
                        h_grp[g][:], ps[:], act_fn, bias=bias, scale=1.0
                    )

                if g >= SKEW:
                    gp = g - SKEW
                    tp = gp // G_PER_TILE
                    offp = (gp % G_PER_TILE) * GRP
                    ps = ps_grp[gp]
                    for c0, c1, dec in _mm2_pieces(gp, bnd):
                        w_s = w2p_s if dec else w2_s
                        nc.tensor.matmul(
                            ps[:, c0:c1], w_s, h_grp[gp][:, c0:c1],
                            start=True, stop=True,
                        )
                    nc.vector.tensor_copy(
                        y_tiles[tp][:, offp:offp + GRP], ps[:]
                    )
                    last_tile = tp == n_tiles - 1
                    d0 = tp * TILE_TOK
                    if last_tile and gp % G_PER_TILE == 1:
                        # first half of the final tile drains early (gp ring)
                        nc.gpsimd.dma_start(
                            y_d[:, d0:d0 + 2 * GRP], y_tiles[tp][:, 0:2 * GRP]
                        )
                    elif last_tile and gp % G_PER_TILE == 3:
                        # final half on the sync ring (its input work is done)
                        nc.sync.dma_start(
                            y_d[:, d0 + 2 * GRP:d0 + TILE_TOK],
                            y_tiles[tp][:, 2 * GRP:TILE_TOK],
                        )
                    elif gp % G_PER_TILE == G_PER_TILE - 1:
                        nc.gpsimd.dma_start(
                            y_d[:, d0:d0 + TILE_TOK], y_tiles[tp][:]
                        )

    nc.finalize()
    return nc


def _get_nc(n_tiles, bnd, has_bias):
    key = ("nc", n_tiles, bnd, has_bias)
    if key not in _CACHE:
        _CACHE[key] = _build_nc(n_tiles, bnd, has_bias)
    return _CACHE[key]


def kernel(x, mask, W1, b1, W2, b2, _trace=False):
    from ml_dtypes import bfloat16, float8_e3m4
    from concourse.bass_utils import run_bass_kernel_spmd

    x = np.asarray(x, dtype=np.float32)
    mask = np.asarray(mask)
    W1b = np.ascontiguousarray(np.asarray(W1, dtype=np.float32)).astype(bfloat16)
    W2f = np.ascontiguousarray(np.asarray(W2, dtype=np.float32))
    W2b = W2f.astype(bfloat16)
    W2pb = (W2f * DECAY_FACTOR).astype(bfloat16)
    wp = np.concatenate([W1b, W2b, W2pb], axis=1)  # [128, 384]
    b1v = np.asarray(b1, dtype=np.float32).reshape(D, 1)
    has_bias = bool(np.any(b1v))
    b2 = np.asarray(b2, dtype=np.float32)

    t = np.arange(S)
    dec_frame = (t + 1) % RESET_PERIOD == 0

    mask_flat = mask.reshape(-1)
    dec_flat = np.broadcast_to(dec_frame[None, :], (B, S)).reshape(-1)
    idx = np.flatnonzero(mask_flat)
    K = idx.size
    out_flat = np.zeros((B * S, D), dtype=np.float32)
    if K:
        sel_dec = dec_flat[idx]
        idx_norm = idx[~sel_dec]
        idx_dec = idx[sel_dec]
        n_norm = -(-idx_norm.size // N_CORES)
        n_dec = -(-idx_dec.size // N_CORES)
        bnd = n_norm
        t_req = n_norm + n_dec
        n_tiles = max(1, -(-t_req // TILE_TOK))
        T = n_tiles * TILE_TOK

        # per-core slot -> source token index (-1 = padding)
        src = np.full((N_CORES, T), -1, dtype=np.int64)
        for c in range(N_CORES):
            a = idx_norm[c * n_norm:(c + 1) * n_norm]
            src[c, :a.size] = a
            d = idx_dec[c * n_dec:(c + 1) * n_dec]
            src[c, bnd:bnd + d.size] = d
        valid = src >= 0

        xp = np.zeros((N_CORES, T, D), dtype=np.float32)
        xp[valid] = x.reshape(B * S, D)[src[valid]]
        x8 = xp.astype(float8_e3m4)
        # feature-major: [core, 128(d), T]
        x_t_all = np.ascontiguousarray(x8.transpose(0, 2, 1))

        in_maps = []
        for c in range(N_CORES):
            m = {"x_t": x_t_all[c], "wp": wp}
            if has_bias:
                m["b1"] = b1v
            in_maps.append(m)

        nc = _get_nc(n_tiles, bnd, has_bias)
        res = run_bass_kernel_spmd(nc, in_maps, list(range(N_CORES)), trace=_trace)
        if _trace:
            _CACHE["last_results"] = res
        yp = np.stack(
            [np.asarray(res.results[c]["y_t"]) for c in range(N_CORES)]
        )  # [cores, 128, T] bf16
        yp = yp.transpose(0, 2, 1).astype(np.float32)  # [cores, T, 128]
        out_flat[src[valid]] = yp[valid]

    out = out_flat.reshape(B, S, D)
    if np.any(b2):
        # device computes h @ W2(/W2'); the masked/decayed bias lands here
        scale = np.where(dec_frame, DECAY_FACTOR, 1.0).astype(np.float32)
        s = mask.astype(np.float32) * scale[None, :]
        out = out + s[:, :, None] * b2[None, None, :]
    return out
